# revision 9
# baseline (speedup 1.0000x reference)
"""GQA multi-head attention (RoPE + tanh softcap + causal mask) on 8 TRN2 cores.

Sharding: tensor-parallel over the 8 kv-head groups (1 kv head + its 4 q heads
per core).  Each core computes its Q/K/V projections from the full hidden
states, runs attention for its 4 q heads, and produces a partial output
through its row-slice of Wo; the host sums the 8 partials.

v2 layout/schedule (vs the fp32 two-phase baseline):
  - all matmul operands are bf16 (PSUM accumulation stays fp32; softmax
    logits/tanh stay fp32).  Halves DMA + SBUF traffic and doubles DVE
    throughput on 16-bit elementwise work.  Measured end-to-end rel err
    ~4e-3 vs the 2e-2 gate.
  - single fused per-block pipeline: project block n (two 3-output passes
    over resident hs tiles) -> attention for q-block n over kv chunks
    0..n -> output projection rows of block n.  The tensor engine always
    has matmul work queued, so the HAM clock gate stays at 8/8 (the old
    kernel ran at 4/8 for 75% of its span).
  - softmax denominators accumulate on the PE: a per-chunk [1,512]
    ones-matmul rides the same PSUM accumulation pattern as A@V, replacing
    the serial vector-engine running-sum chain.
  - 1/denominator via the custom-DVE reciprocal_approx_fast (~5x faster
    than the 8-cycle/element iterative divide).
  - V tiles are transposed with the DMA crossbar (dma_start_transpose)
    instead of PE transposes, freeing PE time and a PSUM bank.
"""

import numpy as np

S, D, DH = 2048, 4096, 128
HQ, HKV = 32, 8
G = HQ // HKV            # q heads per core
N_CORES = 8
MULT = 0.08838834764831845
SOFTCAP = 30.0
ROPE_BASE = 10000.0
BLK = 512                # seq block
NB = S // BLK            # 4 seq blocks
NCH = S // 128           # 16 kcol chunks
NDC = D // 128           # 32 contraction chunks for projections
HDC = NDC // 2           # 16 d-chunks per hs half-block tile

_CACHE = {}


def _classify_mask(mask):
    """Per (qblock, kchunk): skip (all masked), plain (all visible), or
    mixed (transposed {0,1} tile, deduped).  active[n] = ordered
    [(chunk, slot)], slot -1 for plain; mtiles packed [n_uniq*128, BLK]."""
    m = np.asarray(mask).reshape(S, S)
    active = []
    mtiles = []
    seen = {}
    for n in range(NB):
        rows = m[n * BLK:(n + 1) * BLK]
        lst = []
        for c in range(NCH):
            sub = rows[:, c * 128:(c + 1) * 128]
            if not sub.any():
                continue
            if sub.all():
                lst.append((c, -1))
            else:
                t = np.ascontiguousarray(sub.T).astype(np.float32)
                key = t.tobytes()
                if key not in seen:
                    seen[key] = len(mtiles)
                    mtiles.append(t)
                lst.append((c, seen[key]))
        active.append(lst)
    mt = (np.concatenate([t.reshape(128, BLK) for t in mtiles], axis=0)
          if mtiles else None)
    return active, mt


def _build(active, n_uniq):
    import concourse.bacc as bacc
    import concourse.mybir as mybir
    from concourse import tile
    from contextlib import ExitStack

    fp32 = mybir.dt.float32
    bf16 = mybir.dt.bfloat16
    AF = mybir.ActivationFunctionType

    nc = bacc.Bacc("TRN2", target_bir_lowering=False, debug=False,
                   enable_asserts=True, num_devices=N_CORES)
    hsT = nc.dram_tensor("hsT", [D, S], bf16, kind="ExternalInput").ap()
    wq = nc.dram_tensor("wq", [D, G * DH], bf16, kind="ExternalInput").ap()
    wk = nc.dram_tensor("wk", [D, DH], bf16, kind="ExternalInput").ap()
    wv = nc.dram_tensor("wv", [D, DH], bf16, kind="ExternalInput").ap()
    wo = nc.dram_tensor("wo", [G * DH, D], bf16, kind="ExternalInput").ap()
    cosT = nc.dram_tensor("cosT", [DH, S], bf16, kind="ExternalInput").ap()
    sinT = nc.dram_tensor("sinT", [DH, S], bf16, kind="ExternalInput").ap()
    maskm = (nc.dram_tensor("maskm", [n_uniq * 128, BLK], bf16,
                            kind="ExternalInput").ap() if n_uniq else None)
    out = nc.dram_tensor("out", [S, D], fp32, kind="ExternalOutput").ap()

    hsT_r = hsT.rearrange("(c p) s -> p c s", p=128)
    wq_r = wq.rearrange("(c p) m -> p c m", p=128)
    wk_r = wk.rearrange("(c p) m -> p c m", p=128)
    wv_r = wv.rearrange("(c p) m -> p c m", p=128)
    wo_r = wo.rearrange("(c p) n -> p c n", p=128)

    with tile.TileContext(nc) as tc, ExitStack() as top:
        persist = top.enter_context(tc.tile_pool(name="persist", bufs=1))
        # weights: per-head wq tiles so the first matmuls don't wait on the
        # whole 4MB load
        wq_sb = [persist.tile([128, NDC, DH], bf16, tag=f"wq{h}",
                              name=f"wq{h}") for h in range(G)]
        wk_sb = persist.tile([128, NDC, DH], bf16, tag="wk", name="wk")
        wv_sb = persist.tile([128, NDC, DH], bf16, tag="wv", name="wv")
        wo_sb = persist.tile([128, G, D], bf16, tag="wo", name="wo_sb")
        cos_sb = persist.tile([DH, S], bf16, tag="cos", name="cos")
        sin_sb = persist.tile([DH, S], bf16, tag="sin", name="sin")
        ones_bf = persist.tile([128, 1], bf16, tag="ones", name="ones")
        mask_sb = (persist.tile([128, n_uniq, BLK], bf16, tag="masks",
                                name="masks") if n_uniq else None)
        qT = [[persist.tile([DH, BLK], bf16, tag=f"qT{h}_{n}",
                            name=f"qT{h}_{n}") for n in range(NB)]
              for h in range(G)]
        kT = [persist.tile([DH, BLK], bf16, tag=f"kT{n}", name=f"kT{n}")
              for n in range(NB)]
        vnat = [persist.tile([128, DH], bf16, tag=f"vnat{c}", name=f"vnat{c}")
                for c in range(NCH)]
        attnT = [[persist.tile([DH, BLK], bf16, tag=f"attnT{h}_{n}",
                               name=f"attnT{h}_{n}") for n in range(NB)]
                 for h in range(G)]

        # weight loads on the HWDGE queues (sync+scalar), in first-use order
        nc.sync.dma_start(wq_sb[0][:], wq_r[:, :, 0 * DH:1 * DH])
        nc.sync.dma_start(wq_sb[1][:], wq_r[:, :, 1 * DH:2 * DH])
        for h in range(2, G):
            nc.scalar.dma_start(wq_sb[h][:], wq_r[:, :, h * DH:(h + 1) * DH])
        nc.scalar.dma_start(wk_sb[:], wk_r[:])
        nc.scalar.dma_start(wv_sb[:], wv_r[:])
        nc.scalar.dma_start(cos_sb[:], cosT[:])
        nc.scalar.dma_start(sin_sb[:], sinT[:])
        nc.vector.memset(ones_bf[:], 1.0)
        if n_uniq:
            mm_r = maskm.rearrange("(u p) s -> p u s", p=128)
            nc.scalar.dma_start(mask_sb[:], mm_r[:])
        for g in range(8):
            nc.gpsimd.dma_start(wo_sb[:, :, g * BLK:(g + 1) * BLK],
                                wo_r[:, :, g * BLK:(g + 1) * BLK])

        # working pools
        hsp = top.enter_context(tc.tile_pool(name="hs", bufs=2))
        pps = top.enter_context(tc.tile_pool(name="projps", bufs=2,
                                             space="PSUM"))
        rawp = top.enter_context(tc.tile_pool(name="raw", bufs=2))
        rotp = top.enter_context(tc.tile_pool(name="rot", bufs=2))
        tmpp = top.enter_context(tc.tile_pool(name="tmp", bufs=2))
        vtp = top.enter_context(tc.tile_pool(name="vt", bufs=2))
        ttp = top.enter_context(tc.tile_pool(name="tt", bufs=2))
        wtp = top.enter_context(tc.tile_pool(name="wt", bufs=2))
        dsp = top.enter_context(tc.tile_pool(name="dns", bufs=2))
        bcp = top.enter_context(tc.tile_pool(name="bc", bufs=2))
        osb = top.enter_context(tc.tile_pool(name="osb", bufs=3))
        qkps = top.enter_context(tc.tile_pool(name="qkps", bufs=2,
                                              space="PSUM"))
        avps = top.enter_context(tc.tile_pool(name="avps", bufs=2,
                                              space="PSUM"))
        dnps = top.enter_context(tc.tile_pool(name="dnps", bufs=1,
                                              space="PSUM"))
        wops = top.enter_context(tc.tile_pool(name="wops", bufs=1,
                                              space="PSUM"))

        def wo_block(n):
            for j in range(BLK // 128):
                s = n * (BLK // 128) + j
                for nn2 in range(D // (2 * BLK)):
                    ot = osb.tile([128, 2 * BLK], fp32, tag="ot", name="ot")
                    for half in range(2):
                        nn = nn2 * 2 + half
                        pso = wops.tile([128, BLK], fp32, tag="wop",
                                        name="wop")
                        for h in range(G):
                            nc.tensor.matmul(
                                pso[:], attnT[h][n][:, j * 128:(j + 1) * 128],
                                wo_sb[:, h, nn * BLK:(nn + 1) * BLK],
                                start=(h == 0), stop=(h == G - 1),
                                skip_group_check=True)
                        nc.vector.tensor_copy(
                            ot[:, half * BLK:(half + 1) * BLK], pso[:])
                    nc.gpsimd.dma_start(
                        out[s * 128:(s + 1) * 128,
                            nn2 * 2 * BLK:(nn2 + 1) * 2 * BLK], ot[:])

        def rope_evict(ps, dest, sl):
            raw = rawp.tile([128, BLK], bf16, tag="raw", name="raw")
            nc.vector.tensor_copy(raw[:], ps[:])
            rot = rotp.tile([128, BLK], bf16, tag="rot", name="rot")
            nc.sync.dma_start(rot[0:64, :], raw[64:128, :])
            nc.sync.dma_start(rot[64:128, :], raw[0:64, :])
            tmp = tmpp.tile([128, BLK], bf16, tag="tmp", name="tmp")
            nc.vector.tensor_mul(tmp[:], raw[:], cos_sb[:, sl])
            nc.vector.tensor_mul(rot[:], rot[:], sin_sb[:, sl])
            nc.vector.tensor_add(dest[:], tmp[:], rot[:])

        def hs_load(n, split=False):
            sl = slice(n * BLK, (n + 1) * BLK)
            hs_t = [hsp.tile([128, HDC, BLK], bf16, tag="hs", name="hs_t")
                    for _ in range(2)]
            for t in range(2):
                eng = nc.scalar if (split and t == 0) else nc.sync
                eng.dma_start(hs_t[t][:],
                              hsT_r[:, t * HDC:(t + 1) * HDC, sl])
            return hs_t

        def proj_pass(hs_t, w0, w1):
            ps0 = pps.tile([128, BLK], fp32, tag="projps", name="projps")
            ps1 = pps.tile([128, BLK], fp32, tag="projps", name="projps")
            for d in range(NDC):
                h_ap = hs_t[d // HDC][:, d % HDC, :]
                nc.tensor.matmul(ps0[:], w0[:, d, :], h_ap,
                                 start=(d == 0), stop=(d == NDC - 1))
                nc.tensor.matmul(ps1[:], w1[:, d, :], h_ap,
                                 start=(d == 0), stop=(d == NDC - 1))
            return ps0, ps1

        def attention(n):
            pairs = [active[n][i:i + 2] for i in range(0, len(active[n]), 2)]
            n_mm = len(active[n])
            for h in range(G):
                av = avps.tile([128, BLK], fp32, tag="av", name="av")
                dn = dnps.tile([1, BLK], fp32, tag="dn", name="dn")
                mm_i = 0
                for pair in pairs:
                    w2 = len(pair) * BLK
                    tt = ttp.tile([128, 2 * BLK], fp32, tag="tt", name="tt")
                    for i, (c, slot) in enumerate(pair):
                        qk = qkps.tile([128, BLK], fp32, tag="qk", name="qk")
                        nc.tensor.matmul(
                            qk[:],
                            kT[c // 4][:, (c % 4) * 128:(c % 4 + 1) * 128],
                            qT[h][n][:], start=True, stop=True)
                        nc.scalar.activation(
                            tt[:, i * BLK:(i + 1) * BLK], qk[:],
                            AF.Tanh, scale=1.0 / SOFTCAP)
                    wt = wtp.tile([128, 2 * BLK], bf16, tag="wt", name="wt")
                    nc.scalar.activation(wt[:, :w2], tt[:, :w2], AF.Exp,
                                         scale=SOFTCAP)
                    for i, (c, slot) in enumerate(pair):
                        if slot >= 0:
                            nc.vector.tensor_mul(
                                wt[:, i * BLK:(i + 1) * BLK],
                                wt[:, i * BLK:(i + 1) * BLK],
                                mask_sb[:, slot, :])
                    for i, (c, slot) in enumerate(pair):
                        wt_ap = wt[:, i * BLK:(i + 1) * BLK]
                        nc.tensor.matmul(av[:], vnat[c][:], wt_ap,
                                         start=(mm_i == 0),
                                         stop=(mm_i == n_mm - 1),
                                         skip_group_check=True)
                        nc.tensor.matmul(dn[:], ones_bf[:], wt_ap,
                                         start=(mm_i == 0),
                                         stop=(mm_i == n_mm - 1),
                                         skip_group_check=True)
                        mm_i += 1
                dns = dsp.tile([1, BLK], fp32, tag="dns", name="dns")
                nc.vector.reciprocal_approx_fast(dns[:], dn[:])
                bc = bcp.tile([128, BLK], fp32, tag="bc", name="bc")
                nc.gpsimd.partition_broadcast(bc[:], dns[:])
                nc.vector.tensor_mul(attnT[h][n][:], av[:], bc[:])

        def proj_block(n):
            sl = slice(n * BLK, (n + 1) * BLK)
            hs_t = hs_load(n, split=(n == 0))
            ps0, ps1 = proj_pass(hs_t, wq_sb[0], wq_sb[1])
            rope_evict(ps0, qT[0][n], sl)
            rope_evict(ps1, qT[1][n], sl)
            ps2, ps3 = proj_pass(hs_t, wq_sb[2], wq_sb[3])
            rope_evict(ps2, qT[2][n], sl)
            rope_evict(ps3, qT[3][n], sl)
            psk, psv = proj_pass(hs_t, wk_sb, wv_sb)
            rope_evict(psk, kT[n], sl)
            # V: evict to bf16, transpose chunks with the DMA crossbar
            vt = vtp.tile([128, BLK], bf16, tag="vt", name="vt")
            nc.vector.tensor_copy(vt[:], psv[:])
            for j in range(BLK // 128):
                c = n * (BLK // 128) + j
                nc.sync.dma_start_transpose(
                    vnat[c][:], vt[:, j * 128:(j + 1) * 128])

        # Schedule: attention(n) directly after its projection (the
        # scheduler overlaps it with proj(n+1)); each block's output
        # projection is emitted one attention later so its matmuls fill the
        # next attention's pipeline gaps.
        for n in range(NB):
            proj_block(n)
            attention(n)
            if n >= 1:
                wo_block(n - 1)
        wo_block(NB - 1)

    nc.compile()
    return nc


def _rope_tables():
    j = np.arange(0, DH, 2, dtype=np.float32)
    inv = np.float32(1.0) / (np.float32(ROPE_BASE) ** (j / np.float32(DH)))
    t = np.arange(S, dtype=np.float32)
    phase = t[:, None] * inv[None, :]          # [S, 64] fp32 like reference
    cos = np.cos(phase).astype(np.float32)     # [S, 64]
    sin = np.sin(phase).astype(np.float32)
    cosT = np.concatenate([cos.T, cos.T], axis=0)              # [128, S]
    sinT = np.concatenate([-sin.T, sin.T], axis=0)             # sign-folded
    return np.ascontiguousarray(cosT), np.ascontiguousarray(sinT)


def _in_maps(hidden_states, mask, Wq, Wk, Wv, Wo):
    import ml_dtypes
    bf = ml_dtypes.bfloat16
    hs = np.asarray(hidden_states, dtype=np.float32).reshape(S, D)
    Wq = np.asarray(Wq, dtype=np.float32)
    Wk = np.asarray(Wk, dtype=np.float32)
    Wv = np.asarray(Wv, dtype=np.float32)
    Wo = np.asarray(Wo, dtype=np.float32)
    active, mt = _classify_mask(mask)
    hsT = np.ascontiguousarray(hs.T.astype(bf))
    cosT, sinT = _rope_tables()
    cosT = cosT.astype(bf)
    sinT = sinT.astype(bf)
    maps = []
    for c in range(N_CORES):
        m = {
            "hsT": hsT,
            "wq": np.ascontiguousarray(
                (Wq[:, c * G * DH:(c + 1) * G * DH]
                 * np.float32(MULT)).astype(bf)),
            "wk": np.ascontiguousarray(Wk[:, c * DH:(c + 1) * DH].astype(bf)),
            "wv": np.ascontiguousarray(Wv[:, c * DH:(c + 1) * DH].astype(bf)),
            "wo": np.ascontiguousarray(
                Wo[c * G * DH:(c + 1) * G * DH, :].astype(bf)),
            "cosT": cosT,
            "sinT": sinT,
        }
        if mt is not None:
            m["maskm"] = np.ascontiguousarray(mt.astype(bf))
        maps.append(m)
    return active, mt, maps


def kernel(hidden_states, mask, Wq, Wk, Wv, Wo):
    from concourse.bass_utils import run_bass_kernel_spmd

    active, mt, maps = _in_maps(hidden_states, mask, Wq, Wk, Wv, Wo)
    key = tuple(tuple(lst) for lst in active)
    if key not in _CACHE:
        _CACHE[key] = _build(active, 0 if mt is None else mt.shape[0] // 128)
    nc = _CACHE[key]

    res = run_bass_kernel_spmd(nc, maps, list(range(N_CORES)))
    acc = np.zeros((S, D), dtype=np.float64)
    for c in range(N_CORES):
        acc += res.results[c]["out"]
    return acc.astype(np.float32).reshape(1, S, D)


# revision 10
# speedup vs baseline: 1.1817x; 1.1817x over previous
"""GQA multi-head attention (RoPE + tanh softcap + causal mask) on 8 TRN2 cores.

Sharding: tensor-parallel over the 8 kv-head groups (1 kv head + its 4 q heads
per core).  Each core computes its Q/K/V projections from the full hidden
states, runs attention for its 4 q heads, and produces a partial output
through its row-slice of Wo; the host sums the 8 partials.

v2 layout/schedule (vs the fp32 two-phase baseline):
  - all matmul operands are bf16 (PSUM accumulation stays fp32; softmax
    logits/tanh stay fp32).  Halves DMA + SBUF traffic and doubles DVE
    throughput on 16-bit elementwise work.  Measured end-to-end rel err
    ~4e-3 vs the 2e-2 gate.
  - single fused per-block pipeline: project block n (two 3-output passes
    over resident hs tiles) -> attention for q-block n over kv chunks
    0..n -> output projection rows of block n.  The tensor engine always
    has matmul work queued, so the HAM clock gate stays at 8/8 (the old
    kernel ran at 4/8 for 75% of its span).
  - softmax denominators accumulate on the PE: a per-chunk [1,512]
    ones-matmul rides the same PSUM accumulation pattern as A@V, replacing
    the serial vector-engine running-sum chain.
  - 1/denominator via the custom-DVE reciprocal_approx_fast (~5x faster
    than the 8-cycle/element iterative divide).
  - V tiles are transposed with the DMA crossbar (dma_start_transpose)
    instead of PE transposes, freeing PE time and a PSUM bank.
"""

import numpy as np

S, D, DH = 2048, 4096, 128
HQ, HKV = 32, 8
G = HQ // HKV            # q heads per core
N_CORES = 8
MULT = 0.08838834764831845
SOFTCAP = 30.0
ROPE_BASE = 10000.0
BLK = 512                # seq block
NB = S // BLK            # 4 seq blocks
NCH = S // 128           # 16 kcol chunks
NDC = D // 128           # 32 contraction chunks for projections
HDC = NDC // 2           # 16 d-chunks per hs half-block tile

_CACHE = {}


def _classify_mask(mask):
    """Per (qblock, kchunk): skip (all masked), plain (all visible), or
    mixed (transposed {0,1} tile, deduped).  active[n] = ordered
    [(chunk, slot)], slot -1 for plain; mtiles packed [n_uniq*128, BLK]."""
    m = np.asarray(mask).reshape(S, S)
    active = []
    mtiles = []
    seen = {}
    for n in range(NB):
        rows = m[n * BLK:(n + 1) * BLK]
        lst = []
        for c in range(NCH):
            sub = rows[:, c * 128:(c + 1) * 128]
            if not sub.any():
                continue
            if sub.all():
                lst.append((c, -1))
            else:
                t = np.ascontiguousarray(sub.T).astype(np.float32)
                key = t.tobytes()
                if key not in seen:
                    seen[key] = len(mtiles)
                    mtiles.append(t)
                lst.append((c, seen[key]))
        active.append(lst)
    mt = (np.concatenate([t.reshape(128, BLK) for t in mtiles], axis=0)
          if mtiles else None)
    return active, mt


def _build(active, n_uniq):
    import concourse.bacc as bacc
    import concourse.mybir as mybir
    from concourse import tile
    from contextlib import ExitStack

    fp32 = mybir.dt.float32
    bf16 = mybir.dt.bfloat16
    AF = mybir.ActivationFunctionType

    nc = bacc.Bacc("TRN2", target_bir_lowering=False, debug=False,
                   enable_asserts=True, num_devices=N_CORES)
    hsT = nc.dram_tensor("hsT", [D, S], bf16, kind="ExternalInput").ap()
    wq = nc.dram_tensor("wq", [D, G * DH], bf16, kind="ExternalInput").ap()
    wk = nc.dram_tensor("wk", [D, DH], bf16, kind="ExternalInput").ap()
    wv = nc.dram_tensor("wv", [D, DH], bf16, kind="ExternalInput").ap()
    wo = nc.dram_tensor("wo", [G * DH, D], bf16, kind="ExternalInput").ap()
    cosT = nc.dram_tensor("cosT", [DH, S], bf16, kind="ExternalInput").ap()
    sinT = nc.dram_tensor("sinT", [DH, S], bf16, kind="ExternalInput").ap()
    maskm = (nc.dram_tensor("maskm", [n_uniq * 128, BLK], bf16,
                            kind="ExternalInput").ap() if n_uniq else None)
    out = nc.dram_tensor("out", [S, D], fp32, kind="ExternalOutput").ap()

    hsT_r = hsT.rearrange("(c p) s -> p c s", p=128)
    wq_r = wq.rearrange("(c p) m -> p c m", p=128)
    wk_r = wk.rearrange("(c p) m -> p c m", p=128)
    wv_r = wv.rearrange("(c p) m -> p c m", p=128)
    wo_r = wo.rearrange("(c p) n -> p c n", p=128)

    with tile.TileContext(nc) as tc, ExitStack() as top:
        persist = top.enter_context(tc.tile_pool(name="persist", bufs=1))
        # weights: per-head wq tiles so the first matmuls don't wait on the
        # whole 4MB load
        wq_sb = [persist.tile([128, NDC, DH], bf16, tag=f"wq{h}",
                              name=f"wq{h}") for h in range(G)]
        wk_sb = persist.tile([128, NDC, DH], bf16, tag="wk", name="wk")
        wv_sb = persist.tile([128, NDC, DH], bf16, tag="wv", name="wv")
        wo_sb = persist.tile([128, G, D], bf16, tag="wo", name="wo_sb")
        cos_sb = persist.tile([DH, S], bf16, tag="cos", name="cos")
        sin_sb = persist.tile([DH, S], bf16, tag="sin", name="sin")
        ones_bf = persist.tile([128, 1], bf16, tag="ones", name="ones")
        mask_sb = (persist.tile([128, n_uniq, BLK], bf16, tag="masks",
                                name="masks") if n_uniq else None)
        qT = [[persist.tile([DH, BLK], bf16, tag=f"qT{h}_{n}",
                            name=f"qT{h}_{n}") for n in range(NB)]
              for h in range(G)]
        kT = [persist.tile([DH, BLK], bf16, tag=f"kT{n}", name=f"kT{n}")
              for n in range(NB)]
        vnat = [persist.tile([128, DH], bf16, tag=f"vnat{c}", name=f"vnat{c}")
                for c in range(NCH)]
        attnT = [[persist.tile([DH, BLK], bf16, tag=f"attnT{h}_{n}",
                               name=f"attnT{h}_{n}") for n in range(NB)]
                 for h in range(G)]

        # weight loads on the HWDGE queues (sync+scalar), in first-use order
        for h in range(2):
            nc.sync.dma_start(wq_sb[h][:], wq_r[:, :, h * DH:(h + 1) * DH])
        for h in range(2, G):
            nc.scalar.dma_start(wq_sb[h][:], wq_r[:, :, h * DH:(h + 1) * DH])
        nc.sync.dma_start(wk_sb[:], wk_r[:])
        nc.sync.dma_start(wv_sb[:], wv_r[:])
        nc.scalar.dma_start(cos_sb[:], cosT[:])
        nc.scalar.dma_start(sin_sb[:], sinT[:])
        nc.vector.memset(ones_bf[:], 1.0)
        if n_uniq:
            mm_r = maskm.rearrange("(u p) s -> p u s", p=128)
            nc.sync.dma_start(mask_sb[:], mm_r[:])
        for g in range(8):
            nc.gpsimd.dma_start(wo_sb[:, :, g * BLK:(g + 1) * BLK],
                                wo_r[:, :, g * BLK:(g + 1) * BLK])

        # working pools
        hsp = top.enter_context(tc.tile_pool(name="hs", bufs=2))
        pps = top.enter_context(tc.tile_pool(name="projps", bufs=2,
                                             space="PSUM"))
        rawp = top.enter_context(tc.tile_pool(name="raw", bufs=2))
        rotp = top.enter_context(tc.tile_pool(name="rot", bufs=2))
        tmpp = top.enter_context(tc.tile_pool(name="tmp", bufs=2))
        vtp = top.enter_context(tc.tile_pool(name="vt", bufs=2))
        ttp = top.enter_context(tc.tile_pool(name="tt", bufs=2))
        wtp = top.enter_context(tc.tile_pool(name="wt", bufs=2))
        dsp = top.enter_context(tc.tile_pool(name="dns", bufs=2))
        bcp = top.enter_context(tc.tile_pool(name="bc", bufs=2))
        osb = top.enter_context(tc.tile_pool(name="osb", bufs=3))
        qkps = top.enter_context(tc.tile_pool(name="qkps", bufs=2,
                                              space="PSUM"))
        avps = top.enter_context(tc.tile_pool(name="avps", bufs=2,
                                              space="PSUM"))
        dnps = top.enter_context(tc.tile_pool(name="dnps", bufs=1,
                                              space="PSUM"))
        wops = top.enter_context(tc.tile_pool(name="wops", bufs=1,
                                              space="PSUM"))

        def wo_block(n):
            for j in range(BLK // 128):
                s = n * (BLK // 128) + j
                for nn2 in range(D // (2 * BLK)):
                    ot = osb.tile([128, 2 * BLK], fp32, tag="ot", name="ot")
                    for half in range(2):
                        nn = nn2 * 2 + half
                        pso = wops.tile([128, BLK], fp32, tag="wop",
                                        name="wop")
                        for h in range(G):
                            nc.tensor.matmul(
                                pso[:], attnT[h][n][:, j * 128:(j + 1) * 128],
                                wo_sb[:, h, nn * BLK:(nn + 1) * BLK],
                                start=(h == 0), stop=(h == G - 1),
                                skip_group_check=True)
                        nc.vector.tensor_copy(
                            ot[:, half * BLK:(half + 1) * BLK], pso[:])
                    nc.gpsimd.dma_start(
                        out[s * 128:(s + 1) * 128,
                            nn2 * 2 * BLK:(nn2 + 1) * 2 * BLK], ot[:])

        def rope_evict(ps, dest, sl):
            raw = rawp.tile([128, BLK], bf16, tag="raw", name="raw")
            nc.scalar.copy(raw[:], ps[:])
            rot = rotp.tile([128, BLK], bf16, tag="rot", name="rot")
            nc.sync.dma_start(rot[0:64, :], raw[64:128, :])
            nc.sync.dma_start(rot[64:128, :], raw[0:64, :])
            tmp = tmpp.tile([128, BLK], bf16, tag="tmp", name="tmp")
            nc.vector.tensor_mul(tmp[:], raw[:], cos_sb[:, sl])
            nc.vector.tensor_mul(rot[:], rot[:], sin_sb[:, sl])
            nc.vector.tensor_add(dest[:], tmp[:], rot[:])

        def hs_load(n, split=False):
            sl = slice(n * BLK, (n + 1) * BLK)
            hs_t = [hsp.tile([128, HDC, BLK], bf16, tag="hs", name="hs_t")
                    for _ in range(2)]
            for t in range(2):
                eng = nc.scalar if (split and t == 0) else nc.sync
                eng.dma_start(hs_t[t][:],
                              hsT_r[:, t * HDC:(t + 1) * HDC, sl])
            return hs_t

        def proj_pass(hs_t, w0, w1):
            ps0 = pps.tile([128, BLK], fp32, tag="projps", name="projps")
            ps1 = pps.tile([128, BLK], fp32, tag="projps", name="projps")
            for d in range(NDC):
                h_ap = hs_t[d // HDC][:, d % HDC, :]
                nc.tensor.matmul(ps0[:], w0[:, d, :], h_ap,
                                 start=(d == 0), stop=(d == NDC - 1))
                nc.tensor.matmul(ps1[:], w1[:, d, :], h_ap,
                                 start=(d == 0), stop=(d == NDC - 1))
            return ps0, ps1

        def attention(n):
            pairs = [active[n][i:i + 2] for i in range(0, len(active[n]), 2)]
            n_mm = len(active[n])
            for h in range(G):
                av = avps.tile([128, BLK], fp32, tag="av", name="av")
                dn = dnps.tile([1, BLK], fp32, tag="dn", name="dn")
                mm_i = 0
                for pair in pairs:
                    w2 = len(pair) * BLK
                    tt = ttp.tile([128, 2 * BLK], fp32, tag="tt", name="tt")
                    for i, (c, slot) in enumerate(pair):
                        qk = qkps.tile([128, BLK], fp32, tag="qk", name="qk")
                        nc.tensor.matmul(
                            qk[:],
                            kT[c // 4][:, (c % 4) * 128:(c % 4 + 1) * 128],
                            qT[h][n][:], start=True, stop=True)
                        nc.scalar.activation(
                            tt[:, i * BLK:(i + 1) * BLK], qk[:],
                            AF.Tanh, scale=1.0 / SOFTCAP)
                    wt = wtp.tile([128, 2 * BLK], bf16, tag="wt", name="wt")
                    nc.scalar.activation(wt[:, :w2], tt[:, :w2], AF.Exp,
                                         scale=SOFTCAP)
                    for i, (c, slot) in enumerate(pair):
                        if slot >= 0:
                            nc.vector.tensor_mul(
                                wt[:, i * BLK:(i + 1) * BLK],
                                wt[:, i * BLK:(i + 1) * BLK],
                                mask_sb[:, slot, :])
                    for i, (c, slot) in enumerate(pair):
                        wt_ap = wt[:, i * BLK:(i + 1) * BLK]
                        nc.tensor.matmul(av[:], vnat[c][:], wt_ap,
                                         start=(mm_i == 0),
                                         stop=(mm_i == n_mm - 1),
                                         skip_group_check=True)
                        nc.tensor.matmul(dn[:], ones_bf[:], wt_ap,
                                         start=(mm_i == 0),
                                         stop=(mm_i == n_mm - 1),
                                         skip_group_check=True)
                        mm_i += 1
                dns = dsp.tile([1, BLK], fp32, tag="dns", name="dns")
                nc.vector.reciprocal_approx_fast(dns[:], dn[:])
                bc = bcp.tile([128, BLK], fp32, tag="bc", name="bc")
                nc.gpsimd.partition_broadcast(bc[:], dns[:])
                nc.vector.tensor_mul(attnT[h][n][:], av[:], bc[:])

        def proj_block(n):
            sl = slice(n * BLK, (n + 1) * BLK)
            hs_t = hs_load(n)
            ps0, ps1 = proj_pass(hs_t, wq_sb[0], wq_sb[1])
            rope_evict(ps0, qT[0][n], sl)
            rope_evict(ps1, qT[1][n], sl)
            ps2, ps3 = proj_pass(hs_t, wq_sb[2], wq_sb[3])
            rope_evict(ps2, qT[2][n], sl)
            rope_evict(ps3, qT[3][n], sl)
            psk, psv = proj_pass(hs_t, wk_sb, wv_sb)
            rope_evict(psk, kT[n], sl)
            # V: evict to bf16, transpose chunks with the DMA crossbar
            vt = vtp.tile([128, BLK], bf16, tag="vt", name="vt")
            nc.scalar.copy(vt[:], psv[:])
            for j in range(BLK // 128):
                c = n * (BLK // 128) + j
                nc.sync.dma_start_transpose(
                    vnat[c][:], vt[:, j * 128:(j + 1) * 128])

        # Schedule: attention(n) directly after its projection (the
        # scheduler overlaps it with proj(n+1)); each block's output
        # projection is emitted one attention later so its matmuls fill the
        # next attention's pipeline gaps.
        for n in range(NB):
            proj_block(n)
            attention(n)
            if n >= 1:
                wo_block(n - 1)
        wo_block(NB - 1)

    nc.compile()
    return nc


def _rope_tables():
    j = np.arange(0, DH, 2, dtype=np.float32)
    inv = np.float32(1.0) / (np.float32(ROPE_BASE) ** (j / np.float32(DH)))
    t = np.arange(S, dtype=np.float32)
    phase = t[:, None] * inv[None, :]          # [S, 64] fp32 like reference
    cos = np.cos(phase).astype(np.float32)     # [S, 64]
    sin = np.sin(phase).astype(np.float32)
    cosT = np.concatenate([cos.T, cos.T], axis=0)              # [128, S]
    sinT = np.concatenate([-sin.T, sin.T], axis=0)             # sign-folded
    return np.ascontiguousarray(cosT), np.ascontiguousarray(sinT)


def _in_maps(hidden_states, mask, Wq, Wk, Wv, Wo):
    import ml_dtypes
    bf = ml_dtypes.bfloat16
    hs = np.asarray(hidden_states, dtype=np.float32).reshape(S, D)
    Wq = np.asarray(Wq, dtype=np.float32)
    Wk = np.asarray(Wk, dtype=np.float32)
    Wv = np.asarray(Wv, dtype=np.float32)
    Wo = np.asarray(Wo, dtype=np.float32)
    active, mt = _classify_mask(mask)
    hsT = np.ascontiguousarray(hs.T.astype(bf))
    cosT, sinT = _rope_tables()
    cosT = cosT.astype(bf)
    sinT = sinT.astype(bf)
    maps = []
    for c in range(N_CORES):
        m = {
            "hsT": hsT,
            "wq": np.ascontiguousarray(
                (Wq[:, c * G * DH:(c + 1) * G * DH]
                 * np.float32(MULT)).astype(bf)),
            "wk": np.ascontiguousarray(Wk[:, c * DH:(c + 1) * DH].astype(bf)),
            "wv": np.ascontiguousarray(Wv[:, c * DH:(c + 1) * DH].astype(bf)),
            "wo": np.ascontiguousarray(
                Wo[c * G * DH:(c + 1) * G * DH, :].astype(bf)),
            "cosT": cosT,
            "sinT": sinT,
        }
        if mt is not None:
            m["maskm"] = np.ascontiguousarray(mt.astype(bf))
        maps.append(m)
    return active, mt, maps


def kernel(hidden_states, mask, Wq, Wk, Wv, Wo):
    from concourse.bass_utils import run_bass_kernel_spmd

    active, mt, maps = _in_maps(hidden_states, mask, Wq, Wk, Wv, Wo)
    key = tuple(tuple(lst) for lst in active)
    if key not in _CACHE:
        _CACHE[key] = _build(active, 0 if mt is None else mt.shape[0] // 128)
    nc = _CACHE[key]

    res = run_bass_kernel_spmd(nc, maps, list(range(N_CORES)))
    acc = np.zeros((S, D), dtype=np.float64)
    for c in range(N_CORES):
        acc += res.results[c]["out"]
    return acc.astype(np.float32).reshape(1, S, D)


# revision 11
# speedup vs baseline: 1.2036x; 1.0186x over previous
"""GQA multi-head attention (RoPE + tanh softcap + causal mask) on 8 TRN2 cores.

Sharding: tensor-parallel over the 8 kv-head groups (1 kv head + its 4 q heads
per core).  Each core computes its Q/K/V projections from the full hidden
states, runs attention for its 4 q heads, and produces a partial output
through its row-slice of Wo; the host sums the 8 partials.

v2 layout/schedule (vs the fp32 two-phase baseline):
  - all matmul operands are bf16 (PSUM accumulation stays fp32; softmax
    logits/tanh stay fp32).  Halves DMA + SBUF traffic and doubles DVE
    throughput on 16-bit elementwise work.  Measured end-to-end rel err
    ~4e-3 vs the 2e-2 gate.
  - single fused per-block pipeline: project block n (two 3-output passes
    over resident hs tiles) -> attention for q-block n over kv chunks
    0..n -> output projection rows of block n.  The tensor engine always
    has matmul work queued, so the HAM clock gate stays at 8/8 (the old
    kernel ran at 4/8 for 75% of its span).
  - softmax denominators accumulate on the PE: a per-chunk [1,512]
    ones-matmul rides the same PSUM accumulation pattern as A@V, replacing
    the serial vector-engine running-sum chain.
  - 1/denominator via the custom-DVE reciprocal_approx_fast (~5x faster
    than the 8-cycle/element iterative divide).
  - V tiles are transposed with the DMA crossbar (dma_start_transpose)
    instead of PE transposes, freeing PE time and a PSUM bank.
"""

import numpy as np

S, D, DH = 2048, 4096, 128
HQ, HKV = 32, 8
G = HQ // HKV            # q heads per core
N_CORES = 8
MULT = 0.08838834764831845
SOFTCAP = 30.0
ROPE_BASE = 10000.0
BLK = 512                # seq block
NB = S // BLK            # 4 seq blocks
NCH = S // 128           # 16 kcol chunks
NDC = D // 128           # 32 contraction chunks for projections
HDC = NDC // 2           # 16 d-chunks per hs half-block tile

_CACHE = {}


def _classify_mask(mask):
    """Per (qblock, kchunk): skip (all masked), plain (all visible), or
    mixed (transposed {0,1} tile, deduped).  active[n] = ordered
    [(chunk, slot)], slot -1 for plain; mtiles packed [n_uniq*128, BLK]."""
    m = np.asarray(mask).reshape(S, S)
    active = []
    mtiles = []
    seen = {}
    for n in range(NB):
        rows = m[n * BLK:(n + 1) * BLK]
        lst = []
        for c in range(NCH):
            sub = rows[:, c * 128:(c + 1) * 128]
            if not sub.any():
                continue
            if sub.all():
                lst.append((c, -1))
            else:
                t = np.ascontiguousarray(sub.T).astype(np.float32)
                key = t.tobytes()
                if key not in seen:
                    seen[key] = len(mtiles)
                    mtiles.append(t)
                lst.append((c, seen[key]))
        active.append(lst)
    mt = (np.concatenate([t.reshape(128, BLK) for t in mtiles], axis=0)
          if mtiles else None)
    return active, mt


def _build(active, n_uniq):
    import concourse.bacc as bacc
    import concourse.mybir as mybir
    from concourse import tile
    from contextlib import ExitStack

    fp32 = mybir.dt.float32
    bf16 = mybir.dt.bfloat16
    AF = mybir.ActivationFunctionType

    nc = bacc.Bacc("TRN2", target_bir_lowering=False, debug=False,
                   enable_asserts=True, num_devices=N_CORES)
    # all inputs host-pre-arranged partition-major so every DMA moves
    # multi-KB contiguous segments per partition (row-major slices of the
    # natural layouts produce 256B-1KB packets and a ~55us DMA-bound start)
    hsT_r = nc.dram_tensor("hsT", [128, NB * 2, HDC, BLK], bf16,
                           kind="ExternalInput").ap()
    wq_r = nc.dram_tensor("wq", [128, G, NDC, DH], bf16,
                          kind="ExternalInput").ap()
    wk_r = nc.dram_tensor("wk", [128, NDC, DH], bf16,
                          kind="ExternalInput").ap()
    wv_r = nc.dram_tensor("wv", [128, NDC, DH], bf16,
                          kind="ExternalInput").ap()
    wo_r = nc.dram_tensor("wo", [128, G, D], bf16,
                          kind="ExternalInput").ap()
    cosT = nc.dram_tensor("cosT", [DH, S], bf16, kind="ExternalInput").ap()
    sinT = nc.dram_tensor("sinT", [DH, S], bf16, kind="ExternalInput").ap()
    maskm = (nc.dram_tensor("maskm", [128, n_uniq, BLK], bf16,
                            kind="ExternalInput").ap() if n_uniq else None)
    out = nc.dram_tensor("out", [S, D], fp32, kind="ExternalOutput").ap()

    with tile.TileContext(nc) as tc, ExitStack() as top:
        persist = top.enter_context(tc.tile_pool(name="persist", bufs=1))
        # weights: per-head wq tiles so the first matmuls don't wait on the
        # whole 4MB load
        wq_sb = [persist.tile([128, NDC, DH], bf16, tag=f"wq{h}",
                              name=f"wq{h}") for h in range(G)]
        wk_sb = persist.tile([128, NDC, DH], bf16, tag="wk", name="wk")
        wv_sb = persist.tile([128, NDC, DH], bf16, tag="wv", name="wv")
        wo_sb = persist.tile([128, G, D], bf16, tag="wo", name="wo_sb")
        cos_sb = persist.tile([DH, S], bf16, tag="cos", name="cos")
        sin_sb = persist.tile([DH, S], bf16, tag="sin", name="sin")
        ones_bf = persist.tile([128, 1], bf16, tag="ones", name="ones")
        mask_sb = (persist.tile([128, n_uniq, BLK], bf16, tag="masks",
                                name="masks") if n_uniq else None)
        qT = [[persist.tile([DH, BLK], bf16, tag=f"qT{h}_{n}",
                            name=f"qT{h}_{n}") for n in range(NB)]
              for h in range(G)]
        kT = [persist.tile([DH, BLK], bf16, tag=f"kT{n}", name=f"kT{n}")
              for n in range(NB)]
        vnat = [persist.tile([128, DH], bf16, tag=f"vnat{c}", name=f"vnat{c}")
                for c in range(NCH)]
        attnT = [[persist.tile([DH, BLK], bf16, tag=f"attnT{h}_{n}",
                               name=f"attnT{h}_{n}") for n in range(NB)]
                 for h in range(G)]

        # weight loads on the HWDGE queues (sync+scalar), in first-use order
        for h in range(2):
            nc.sync.dma_start(wq_sb[h][:], wq_r[:, h])
        for h in range(2, G):
            nc.scalar.dma_start(wq_sb[h][:], wq_r[:, h])
        nc.sync.dma_start(wk_sb[:], wk_r[:])
        nc.sync.dma_start(wv_sb[:], wv_r[:])
        nc.scalar.dma_start(cos_sb[:], cosT[:])
        nc.scalar.dma_start(sin_sb[:], sinT[:])
        nc.vector.memset(ones_bf[:], 1.0)
        if n_uniq:
            nc.sync.dma_start(mask_sb[:], maskm[:])
        for g in range(8):
            nc.gpsimd.dma_start(wo_sb[:, :, g * BLK:(g + 1) * BLK],
                                wo_r[:, :, g * BLK:(g + 1) * BLK])

        # working pools
        hsp = top.enter_context(tc.tile_pool(name="hs", bufs=2))
        pps = top.enter_context(tc.tile_pool(name="projps", bufs=2,
                                             space="PSUM"))
        rawp = top.enter_context(tc.tile_pool(name="raw", bufs=2))
        rotp = top.enter_context(tc.tile_pool(name="rot", bufs=2))
        tmpp = top.enter_context(tc.tile_pool(name="tmp", bufs=2))
        vtp = top.enter_context(tc.tile_pool(name="vt", bufs=2))
        ttp = top.enter_context(tc.tile_pool(name="tt", bufs=2))
        wtp = top.enter_context(tc.tile_pool(name="wt", bufs=2))
        dsp = top.enter_context(tc.tile_pool(name="dns", bufs=2))
        bcp = top.enter_context(tc.tile_pool(name="bc", bufs=2))
        osb = top.enter_context(tc.tile_pool(name="osb", bufs=3))
        qkps = top.enter_context(tc.tile_pool(name="qkps", bufs=2,
                                              space="PSUM"))
        avps = top.enter_context(tc.tile_pool(name="avps", bufs=2,
                                              space="PSUM"))
        dnps = top.enter_context(tc.tile_pool(name="dnps", bufs=1,
                                              space="PSUM"))
        wops = top.enter_context(tc.tile_pool(name="wops", bufs=1,
                                              space="PSUM"))

        def wo_block(n):
            for j in range(BLK // 128):
                s = n * (BLK // 128) + j
                for nn2 in range(D // (2 * BLK)):
                    ot = osb.tile([128, 2 * BLK], fp32, tag="ot", name="ot")
                    for half in range(2):
                        nn = nn2 * 2 + half
                        pso = wops.tile([128, BLK], fp32, tag="wop",
                                        name="wop")
                        for h in range(G):
                            nc.tensor.matmul(
                                pso[:], attnT[h][n][:, j * 128:(j + 1) * 128],
                                wo_sb[:, h, nn * BLK:(nn + 1) * BLK],
                                start=(h == 0), stop=(h == G - 1),
                                skip_group_check=True)
                        nc.vector.tensor_copy(
                            ot[:, half * BLK:(half + 1) * BLK], pso[:])
                    nc.gpsimd.dma_start(
                        out[s * 128:(s + 1) * 128,
                            nn2 * 2 * BLK:(nn2 + 1) * 2 * BLK], ot[:])

        def rope_evict(ps, dest, sl):
            raw = rawp.tile([128, BLK], bf16, tag="raw", name="raw")
            nc.scalar.copy(raw[:], ps[:])
            rot = rotp.tile([128, BLK], bf16, tag="rot", name="rot")
            nc.sync.dma_start(rot[0:64, :], raw[64:128, :])
            nc.sync.dma_start(rot[64:128, :], raw[0:64, :])
            tmp = tmpp.tile([128, BLK], bf16, tag="tmp", name="tmp")
            nc.vector.tensor_mul(tmp[:], raw[:], cos_sb[:, sl])
            nc.vector.tensor_mul(rot[:], rot[:], sin_sb[:, sl])
            nc.vector.tensor_add(dest[:], tmp[:], rot[:])

        def hs_load(n, split=False):
            hs_t = [hsp.tile([128, HDC, BLK], bf16, tag="hs", name="hs_t")
                    for _ in range(2)]
            for t in range(2):
                eng = nc.scalar if (split and t == 0) else nc.sync
                eng.dma_start(hs_t[t][:], hsT_r[:, n * 2 + t])
            return hs_t

        def proj_pass(hs_t, w0, w1):
            ps0 = pps.tile([128, BLK], fp32, tag="projps", name="projps")
            ps1 = pps.tile([128, BLK], fp32, tag="projps", name="projps")
            for d in range(NDC):
                h_ap = hs_t[d // HDC][:, d % HDC, :]
                nc.tensor.matmul(ps0[:], w0[:, d, :], h_ap,
                                 start=(d == 0), stop=(d == NDC - 1))
                nc.tensor.matmul(ps1[:], w1[:, d, :], h_ap,
                                 start=(d == 0), stop=(d == NDC - 1))
            return ps0, ps1

        def attention(n):
            pairs = [active[n][i:i + 2] for i in range(0, len(active[n]), 2)]
            n_mm = len(active[n])
            for h in range(G):
                av = avps.tile([128, BLK], fp32, tag="av", name="av")
                dn = dnps.tile([1, BLK], fp32, tag="dn", name="dn")
                mm_i = 0
                for pair in pairs:
                    w2 = len(pair) * BLK
                    tt = ttp.tile([128, 2 * BLK], fp32, tag="tt", name="tt")
                    for i, (c, slot) in enumerate(pair):
                        qk = qkps.tile([128, BLK], fp32, tag="qk", name="qk")
                        nc.tensor.matmul(
                            qk[:],
                            kT[c // 4][:, (c % 4) * 128:(c % 4 + 1) * 128],
                            qT[h][n][:], start=True, stop=True)
                        nc.scalar.activation(
                            tt[:, i * BLK:(i + 1) * BLK], qk[:],
                            AF.Tanh, scale=1.0 / SOFTCAP)
                    wt = wtp.tile([128, 2 * BLK], bf16, tag="wt", name="wt")
                    nc.scalar.activation(wt[:, :w2], tt[:, :w2], AF.Exp,
                                         scale=SOFTCAP)
                    for i, (c, slot) in enumerate(pair):
                        if slot >= 0:
                            nc.vector.tensor_mul(
                                wt[:, i * BLK:(i + 1) * BLK],
                                wt[:, i * BLK:(i + 1) * BLK],
                                mask_sb[:, slot, :])
                    for i, (c, slot) in enumerate(pair):
                        wt_ap = wt[:, i * BLK:(i + 1) * BLK]
                        nc.tensor.matmul(av[:], vnat[c][:], wt_ap,
                                         start=(mm_i == 0),
                                         stop=(mm_i == n_mm - 1),
                                         skip_group_check=True)
                        nc.tensor.matmul(dn[:], ones_bf[:], wt_ap,
                                         start=(mm_i == 0),
                                         stop=(mm_i == n_mm - 1),
                                         skip_group_check=True)
                        mm_i += 1
                dns = dsp.tile([1, BLK], fp32, tag="dns", name="dns")
                nc.vector.reciprocal_approx_fast(dns[:], dn[:])
                bc = bcp.tile([128, BLK], fp32, tag="bc", name="bc")
                nc.gpsimd.partition_broadcast(bc[:], dns[:])
                nc.vector.tensor_mul(attnT[h][n][:], av[:], bc[:])

        def proj_block(n):
            sl = slice(n * BLK, (n + 1) * BLK)
            hs_t = hs_load(n)
            ps0, ps1 = proj_pass(hs_t, wq_sb[0], wq_sb[1])
            rope_evict(ps0, qT[0][n], sl)
            rope_evict(ps1, qT[1][n], sl)
            ps2, ps3 = proj_pass(hs_t, wq_sb[2], wq_sb[3])
            rope_evict(ps2, qT[2][n], sl)
            rope_evict(ps3, qT[3][n], sl)
            psk, psv = proj_pass(hs_t, wk_sb, wv_sb)
            rope_evict(psk, kT[n], sl)
            # V: evict to bf16, transpose chunks with the DMA crossbar
            vt = vtp.tile([128, BLK], bf16, tag="vt", name="vt")
            nc.scalar.copy(vt[:], psv[:])
            for j in range(BLK // 128):
                c = n * (BLK // 128) + j
                nc.sync.dma_start_transpose(
                    vnat[c][:], vt[:, j * 128:(j + 1) * 128])

        # Schedule: attention(n) directly after its projection (the
        # scheduler overlaps it with proj(n+1)); each block's output
        # projection is emitted one attention later so its matmuls fill the
        # next attention's pipeline gaps.
        for n in range(NB):
            proj_block(n)
            attention(n)
            if n >= 1:
                wo_block(n - 1)
        wo_block(NB - 1)

    nc.compile()
    return nc


def _rope_tables():
    j = np.arange(0, DH, 2, dtype=np.float32)
    inv = np.float32(1.0) / (np.float32(ROPE_BASE) ** (j / np.float32(DH)))
    t = np.arange(S, dtype=np.float32)
    phase = t[:, None] * inv[None, :]          # [S, 64] fp32 like reference
    cos = np.cos(phase).astype(np.float32)     # [S, 64]
    sin = np.sin(phase).astype(np.float32)
    cosT = np.concatenate([cos.T, cos.T], axis=0)              # [128, S]
    sinT = np.concatenate([-sin.T, sin.T], axis=0)             # sign-folded
    return np.ascontiguousarray(cosT), np.ascontiguousarray(sinT)


def _in_maps(hidden_states, mask, Wq, Wk, Wv, Wo):
    import ml_dtypes
    bf = ml_dtypes.bfloat16
    hs = np.asarray(hidden_states, dtype=np.float32).reshape(S, D)
    Wq = np.asarray(Wq, dtype=np.float32)
    Wk = np.asarray(Wk, dtype=np.float32)
    Wv = np.asarray(Wv, dtype=np.float32)
    Wo = np.asarray(Wo, dtype=np.float32)
    active, mt = _classify_mask(mask)
    # partition-major pre-arrangements (see _build comment): index d becomes
    # (chunk c, partition p) with p fastest; tiles made contiguous
    hsT = hs.T.astype(bf)                       # [D, S]
    hs_pre = np.ascontiguousarray(              # [128, NB*2, HDC, BLK]
        hsT.reshape(2, HDC, 128, NB, BLK).transpose(2, 3, 0, 1, 4)
        .reshape(128, NB * 2, HDC, BLK))
    cosT, sinT = _rope_tables()
    cosT = cosT.astype(bf)
    sinT = sinT.astype(bf)
    mask_pre = (np.ascontiguousarray(
        mt.astype(bf).reshape(-1, 128, BLK).transpose(1, 0, 2))
        if mt is not None else None)
    maps = []
    for c in range(N_CORES):
        wq_c = (Wq[:, c * G * DH:(c + 1) * G * DH]
                * np.float32(MULT)).astype(bf)          # [D, G*DH]
        wk_c = Wk[:, c * DH:(c + 1) * DH].astype(bf)    # [D, DH]
        wv_c = Wv[:, c * DH:(c + 1) * DH].astype(bf)
        wo_c = Wo[c * G * DH:(c + 1) * G * DH, :].astype(bf)  # [G*DH, D]
        m = {
            "hsT": hs_pre,
            "wq": np.ascontiguousarray(                 # [128, G, NDC, DH]
                wq_c.reshape(NDC, 128, G, DH).transpose(1, 2, 0, 3)),
            "wk": np.ascontiguousarray(                 # [128, NDC, DH]
                wk_c.reshape(NDC, 128, DH).transpose(1, 0, 2)),
            "wv": np.ascontiguousarray(
                wv_c.reshape(NDC, 128, DH).transpose(1, 0, 2)),
            "wo": np.ascontiguousarray(                 # [128, G, D]
                wo_c.reshape(G, 128, D).transpose(1, 0, 2)),
            "cosT": cosT,
            "sinT": sinT,
        }
        if mask_pre is not None:
            m["maskm"] = mask_pre
        maps.append(m)
    return active, mt, maps


def kernel(hidden_states, mask, Wq, Wk, Wv, Wo):
    from concourse.bass_utils import run_bass_kernel_spmd

    active, mt, maps = _in_maps(hidden_states, mask, Wq, Wk, Wv, Wo)
    key = tuple(tuple(lst) for lst in active)
    if key not in _CACHE:
        _CACHE[key] = _build(active, 0 if mt is None else mt.shape[0] // 128)
    nc = _CACHE[key]

    res = run_bass_kernel_spmd(nc, maps, list(range(N_CORES)))
    acc = np.zeros((S, D), dtype=np.float64)
    for c in range(N_CORES):
        acc += res.results[c]["out"]
    return acc.astype(np.float32).reshape(1, S, D)


# revision 12
# speedup vs baseline: 1.2854x; 1.0679x over previous
"""GQA multi-head attention (RoPE + tanh softcap + causal mask) on 8 TRN2 cores.

Sharding: tensor-parallel over the 8 kv-head groups (1 kv head + its 4 q heads
per core).  Each core computes its Q/K/V projections from the full hidden
states, runs attention for its 4 q heads, and produces a partial output
through its row-slice of Wo; the host sums the 8 partials.

v2 layout/schedule (vs the fp32 two-phase baseline):
  - all matmul operands are bf16 (PSUM accumulation stays fp32; softmax
    logits/tanh stay fp32).  Halves DMA + SBUF traffic and doubles DVE
    throughput on 16-bit elementwise work.  Measured end-to-end rel err
    ~4e-3 vs the 2e-2 gate.
  - single fused per-block pipeline: project block n (two 3-output passes
    over resident hs tiles) -> attention for q-block n over kv chunks
    0..n -> output projection rows of block n.  The tensor engine always
    has matmul work queued, so the HAM clock gate stays at 8/8 (the old
    kernel ran at 4/8 for 75% of its span).
  - softmax denominators accumulate on the PE: a per-chunk [1,512]
    ones-matmul rides the same PSUM accumulation pattern as A@V, replacing
    the serial vector-engine running-sum chain.
  - 1/denominator via the custom-DVE reciprocal_approx_fast (~5x faster
    than the 8-cycle/element iterative divide).
  - V tiles are transposed with the DMA crossbar (dma_start_transpose)
    instead of PE transposes, freeing PE time and a PSUM bank.
"""

import numpy as np

S, D, DH = 2048, 4096, 128
HQ, HKV = 32, 8
G = HQ // HKV            # q heads per core
N_CORES = 8
MULT = 0.08838834764831845
SOFTCAP = 30.0
ROPE_BASE = 10000.0
BLK = 512                # seq block
NB = S // BLK            # 4 seq blocks
NCH = S // 128           # 16 kcol chunks
NDC = D // 128           # 32 contraction chunks for projections
HDC = NDC // 2           # 16 d-chunks per hs half-block tile

_CACHE = {}


def _classify_mask(mask):
    """Per (qblock, kchunk): skip (all masked), plain (all visible), or
    mixed (transposed {0,1} tile, deduped).  active[n] = ordered
    [(chunk, slot)], slot -1 for plain; mtiles packed [n_uniq*128, BLK]."""
    m = np.asarray(mask).reshape(S, S)
    active = []
    mtiles = []
    seen = {}
    for n in range(NB):
        rows = m[n * BLK:(n + 1) * BLK]
        lst = []
        for c in range(NCH):
            sub = rows[:, c * 128:(c + 1) * 128]
            if not sub.any():
                continue
            if sub.all():
                lst.append((c, -1))
            else:
                t = np.ascontiguousarray(sub.T).astype(np.float32)
                key = t.tobytes()
                if key not in seen:
                    seen[key] = len(mtiles)
                    mtiles.append(t)
                lst.append((c, seen[key]))
        active.append(lst)
    mt = (np.concatenate([t.reshape(128, BLK) for t in mtiles], axis=0)
          if mtiles else None)
    return active, mt


def _build(active, n_uniq):
    import concourse.bacc as bacc
    import concourse.mybir as mybir
    from concourse import tile
    from contextlib import ExitStack

    fp32 = mybir.dt.float32
    bf16 = mybir.dt.bfloat16
    AF = mybir.ActivationFunctionType

    nc = bacc.Bacc("TRN2", target_bir_lowering=False, debug=False,
                   enable_asserts=True, num_devices=N_CORES)
    # all inputs host-pre-arranged partition-major so every DMA moves
    # multi-KB contiguous segments per partition (row-major slices of the
    # natural layouts produce 256B-1KB packets and a ~55us DMA-bound start)
    hsT_r = nc.dram_tensor("hsT", [128, NB * 2, HDC, BLK], bf16,
                           kind="ExternalInput").ap()
    wq_r = nc.dram_tensor("wq", [128, G, NDC, DH], bf16,
                          kind="ExternalInput").ap()
    wk_r = nc.dram_tensor("wk", [128, NDC, DH], bf16,
                          kind="ExternalInput").ap()
    wv_r = nc.dram_tensor("wv", [128, NDC, DH], bf16,
                          kind="ExternalInput").ap()
    wo_r = nc.dram_tensor("wo", [128, G, D], bf16,
                          kind="ExternalInput").ap()
    cosT = nc.dram_tensor("cosT", [DH, S], bf16, kind="ExternalInput").ap()
    sinT = nc.dram_tensor("sinT", [DH, S], bf16, kind="ExternalInput").ap()
    maskm = (nc.dram_tensor("maskm", [128, n_uniq, BLK], bf16,
                            kind="ExternalInput").ap() if n_uniq else None)
    out = nc.dram_tensor("out", [S, D], fp32, kind="ExternalOutput").ap()

    with tile.TileContext(nc) as tc, ExitStack() as top:
        persist = top.enter_context(tc.tile_pool(name="persist", bufs=1))
        # weights: per-head wq tiles so the first matmuls don't wait on the
        # whole 4MB load
        wq_sb = [persist.tile([128, NDC, DH], bf16, tag=f"wq{h}",
                              name=f"wq{h}") for h in range(G)]
        wk_sb = persist.tile([128, NDC, DH], bf16, tag="wk", name="wk")
        wv_sb = persist.tile([128, NDC, DH], bf16, tag="wv", name="wv")
        wo_sb = persist.tile([128, G, D], bf16, tag="wo", name="wo_sb")
        cos_sb = persist.tile([DH, S], bf16, tag="cos", name="cos")
        sin_sb = persist.tile([DH, S], bf16, tag="sin", name="sin")
        ones_bf = persist.tile([128, 1], bf16, tag="ones", name="ones")
        mask_sb = (persist.tile([128, n_uniq, BLK], bf16, tag="masks",
                                name="masks") if n_uniq else None)
        qT = [[persist.tile([DH, BLK], bf16, tag=f"qT{h}_{n}",
                            name=f"qT{h}_{n}") for n in range(NB)]
              for h in range(G)]
        kT = [persist.tile([DH, BLK], bf16, tag=f"kT{n}", name=f"kT{n}")
              for n in range(NB)]
        vnat = [persist.tile([128, DH], bf16, tag=f"vnat{c}", name=f"vnat{c}")
                for c in range(NCH)]
        attnT = [[persist.tile([DH, BLK], bf16, tag=f"attnT{h}_{n}",
                               name=f"attnT{h}_{n}") for n in range(NB)]
                 for h in range(G)]

        # weight loads: only wq0/wq1 (+ the first hs tiles, loaded in
        # proj_block) are needed immediately; defer the rest with staggered
        # scheduler timestamps so the critical first loads get the full DMA
        # bandwidth instead of fair-sharing with a 13MB flood
        for h in range(2):
            nc.sync.dma_start(wq_sb[h][:], wq_r[:, h])
        nc.vector.memset(ones_bf[:], 1.0)
        with tc.tile_wait_until(0.008):
            for h in range(2, G):
                nc.scalar.dma_start(wq_sb[h][:], wq_r[:, h])
        with tc.tile_wait_until(0.016):
            nc.sync.dma_start(wk_sb[:], wk_r[:])
            nc.sync.dma_start(wv_sb[:], wv_r[:])
            nc.scalar.dma_start(cos_sb[:], cosT[:])
            nc.scalar.dma_start(sin_sb[:], sinT[:])
        with tc.tile_wait_until(0.028):
            if n_uniq:
                nc.scalar.dma_start(mask_sb[:], maskm[:])
        with tc.tile_wait_until(0.036):
            for g in range(8):
                nc.gpsimd.dma_start(wo_sb[:, :, g * BLK:(g + 1) * BLK],
                                    wo_r[:, :, g * BLK:(g + 1) * BLK])

        # working pools
        hsp = top.enter_context(tc.tile_pool(name="hs", bufs=2))
        pps = top.enter_context(tc.tile_pool(name="projps", bufs=2,
                                             space="PSUM"))
        rawp = top.enter_context(tc.tile_pool(name="raw", bufs=2))
        rotp = top.enter_context(tc.tile_pool(name="rot", bufs=2))
        tmpp = top.enter_context(tc.tile_pool(name="tmp", bufs=2))
        vtp = top.enter_context(tc.tile_pool(name="vt", bufs=2))
        ttp = top.enter_context(tc.tile_pool(name="tt", bufs=2))
        wtp = top.enter_context(tc.tile_pool(name="wt", bufs=2))
        dsp = top.enter_context(tc.tile_pool(name="dns", bufs=2))
        bcp = top.enter_context(tc.tile_pool(name="bc", bufs=2))
        osb = top.enter_context(tc.tile_pool(name="osb", bufs=3))
        qkps = top.enter_context(tc.tile_pool(name="qkps", bufs=2,
                                              space="PSUM"))
        avps = top.enter_context(tc.tile_pool(name="avps", bufs=1,
                                              space="PSUM"))
        dnps = top.enter_context(tc.tile_pool(name="dnps", bufs=1,
                                              space="PSUM"))
        wops = top.enter_context(tc.tile_pool(name="wops", bufs=2,
                                              space="PSUM"))

        def wo_block(n):
            for j in range(BLK // 128):
                s = n * (BLK // 128) + j
                for nn2 in range(D // (2 * BLK)):
                    ot = osb.tile([128, 2 * BLK], fp32, tag="ot", name="ot")
                    for half in range(2):
                        nn = nn2 * 2 + half
                        pso = wops.tile([128, BLK], fp32, tag="wop",
                                        name="wop")
                        for h in range(G):
                            nc.tensor.matmul(
                                pso[:], attnT[h][n][:, j * 128:(j + 1) * 128],
                                wo_sb[:, h, nn * BLK:(nn + 1) * BLK],
                                start=(h == 0), stop=(h == G - 1),
                                skip_group_check=True)
                        nc.vector.tensor_copy(
                            ot[:, half * BLK:(half + 1) * BLK], pso[:])
                    nc.gpsimd.dma_start(
                        out[s * 128:(s + 1) * 128,
                            nn2 * 2 * BLK:(nn2 + 1) * 2 * BLK], ot[:])

        def rope_evict(ps, dest, sl):
            raw = rawp.tile([128, BLK], bf16, tag="raw", name="raw")
            nc.scalar.copy(raw[:], ps[:])
            rot = rotp.tile([128, BLK], bf16, tag="rot", name="rot")
            nc.sync.dma_start(rot[0:64, :], raw[64:128, :])
            nc.sync.dma_start(rot[64:128, :], raw[0:64, :])
            tmp = tmpp.tile([128, BLK], bf16, tag="tmp", name="tmp")
            nc.vector.tensor_mul(tmp[:], raw[:], cos_sb[:, sl])
            nc.vector.tensor_mul(rot[:], rot[:], sin_sb[:, sl])
            nc.vector.tensor_add(dest[:], tmp[:], rot[:])

        def hs_load(n, split=False):
            hs_t = [hsp.tile([128, HDC, BLK], bf16, tag="hs", name="hs_t")
                    for _ in range(2)]
            for t in range(2):
                eng = nc.scalar if (split and t == 0) else nc.sync
                eng.dma_start(hs_t[t][:], hsT_r[:, n * 2 + t])
            return hs_t

        def proj_pass(hs_t, w0, w1):
            ps0 = pps.tile([128, BLK], fp32, tag="projps", name="projps")
            ps1 = pps.tile([128, BLK], fp32, tag="projps", name="projps")
            for d in range(NDC):
                h_ap = hs_t[d // HDC][:, d % HDC, :]
                nc.tensor.matmul(ps0[:], w0[:, d, :], h_ap,
                                 start=(d == 0), stop=(d == NDC - 1))
                nc.tensor.matmul(ps1[:], w1[:, d, :], h_ap,
                                 start=(d == 0), stop=(d == NDC - 1))
            return ps0, ps1

        def attention(n):
            pairs = [active[n][i:i + 2] for i in range(0, len(active[n]), 2)]
            n_mm = len(active[n])
            for h in range(G):
                av = avps.tile([128, BLK], fp32, tag="av", name="av")
                dn = dnps.tile([1, BLK], fp32, tag="dn", name="dn")
                mm_i = 0
                for pair in pairs:
                    w2 = len(pair) * BLK
                    tt = ttp.tile([128, 2 * BLK], fp32, tag="tt", name="tt")
                    for i, (c, slot) in enumerate(pair):
                        qk = qkps.tile([128, BLK], fp32, tag="qk", name="qk")
                        nc.tensor.matmul(
                            qk[:],
                            kT[c // 4][:, (c % 4) * 128:(c % 4 + 1) * 128],
                            qT[h][n][:], start=True, stop=True)
                        nc.scalar.activation(
                            tt[:, i * BLK:(i + 1) * BLK], qk[:],
                            AF.Tanh, scale=1.0 / SOFTCAP)
                    wt = wtp.tile([128, 2 * BLK], bf16, tag="wt", name="wt")
                    nc.scalar.activation(wt[:, :w2], tt[:, :w2], AF.Exp,
                                         scale=SOFTCAP)
                    for i, (c, slot) in enumerate(pair):
                        if slot >= 0:
                            nc.vector.tensor_mul(
                                wt[:, i * BLK:(i + 1) * BLK],
                                wt[:, i * BLK:(i + 1) * BLK],
                                mask_sb[:, slot, :])
                    for i, (c, slot) in enumerate(pair):
                        wt_ap = wt[:, i * BLK:(i + 1) * BLK]
                        nc.tensor.matmul(av[:], vnat[c][:], wt_ap,
                                         start=(mm_i == 0),
                                         stop=(mm_i == n_mm - 1),
                                         skip_group_check=True)
                        nc.tensor.matmul(dn[:], ones_bf[:], wt_ap,
                                         start=(mm_i == 0),
                                         stop=(mm_i == n_mm - 1),
                                         skip_group_check=True)
                        mm_i += 1
                dns = dsp.tile([1, BLK], fp32, tag="dns", name="dns")
                nc.vector.reciprocal_approx_fast(dns[:], dn[:])
                bc = bcp.tile([128, BLK], fp32, tag="bc", name="bc")
                nc.gpsimd.partition_broadcast(bc[:], dns[:])
                nc.vector.tensor_mul(attnT[h][n][:], av[:], bc[:])

        def proj_block(n):
            sl = slice(n * BLK, (n + 1) * BLK)
            hs_t = hs_load(n)
            ps0, ps1 = proj_pass(hs_t, wq_sb[0], wq_sb[1])
            rope_evict(ps0, qT[0][n], sl)
            rope_evict(ps1, qT[1][n], sl)
            ps2, ps3 = proj_pass(hs_t, wq_sb[2], wq_sb[3])
            rope_evict(ps2, qT[2][n], sl)
            rope_evict(ps3, qT[3][n], sl)
            psk, psv = proj_pass(hs_t, wk_sb, wv_sb)
            rope_evict(psk, kT[n], sl)
            # V: evict to bf16, transpose chunks with the DMA crossbar
            vt = vtp.tile([128, BLK], bf16, tag="vt", name="vt")
            nc.scalar.copy(vt[:], psv[:])
            for j in range(BLK // 128):
                c = n * (BLK // 128) + j
                nc.sync.dma_start_transpose(
                    vnat[c][:], vt[:, j * 128:(j + 1) * 128])

        # Schedule: attention(n) directly after its projection (the
        # scheduler overlaps it with proj(n+1)); each block's output
        # projection is emitted one attention later so its matmuls fill the
        # next attention's pipeline gaps.
        for n in range(NB):
            proj_block(n)
            attention(n)
            if n >= 1:
                wo_block(n - 1)
        wo_block(NB - 1)

    nc.compile()
    return nc


def _rope_tables():
    j = np.arange(0, DH, 2, dtype=np.float32)
    inv = np.float32(1.0) / (np.float32(ROPE_BASE) ** (j / np.float32(DH)))
    t = np.arange(S, dtype=np.float32)
    phase = t[:, None] * inv[None, :]          # [S, 64] fp32 like reference
    cos = np.cos(phase).astype(np.float32)     # [S, 64]
    sin = np.sin(phase).astype(np.float32)
    cosT = np.concatenate([cos.T, cos.T], axis=0)              # [128, S]
    sinT = np.concatenate([-sin.T, sin.T], axis=0)             # sign-folded
    return np.ascontiguousarray(cosT), np.ascontiguousarray(sinT)


def _in_maps(hidden_states, mask, Wq, Wk, Wv, Wo):
    import ml_dtypes
    bf = ml_dtypes.bfloat16
    hs = np.asarray(hidden_states, dtype=np.float32).reshape(S, D)
    Wq = np.asarray(Wq, dtype=np.float32)
    Wk = np.asarray(Wk, dtype=np.float32)
    Wv = np.asarray(Wv, dtype=np.float32)
    Wo = np.asarray(Wo, dtype=np.float32)
    active, mt = _classify_mask(mask)
    # partition-major pre-arrangements (see _build comment): index d becomes
    # (chunk c, partition p) with p fastest; tiles made contiguous
    hsT = hs.T.astype(bf)                       # [D, S]
    hs_pre = np.ascontiguousarray(              # [128, NB*2, HDC, BLK]
        hsT.reshape(2, HDC, 128, NB, BLK).transpose(2, 3, 0, 1, 4)
        .reshape(128, NB * 2, HDC, BLK))
    cosT, sinT = _rope_tables()
    cosT = cosT.astype(bf)
    sinT = sinT.astype(bf)
    mask_pre = (np.ascontiguousarray(
        mt.astype(bf).reshape(-1, 128, BLK).transpose(1, 0, 2))
        if mt is not None else None)
    maps = []
    for c in range(N_CORES):
        wq_c = (Wq[:, c * G * DH:(c + 1) * G * DH]
                * np.float32(MULT)).astype(bf)          # [D, G*DH]
        wk_c = Wk[:, c * DH:(c + 1) * DH].astype(bf)    # [D, DH]
        wv_c = Wv[:, c * DH:(c + 1) * DH].astype(bf)
        wo_c = Wo[c * G * DH:(c + 1) * G * DH, :].astype(bf)  # [G*DH, D]
        m = {
            "hsT": hs_pre,
            "wq": np.ascontiguousarray(                 # [128, G, NDC, DH]
                wq_c.reshape(NDC, 128, G, DH).transpose(1, 2, 0, 3)),
            "wk": np.ascontiguousarray(                 # [128, NDC, DH]
                wk_c.reshape(NDC, 128, DH).transpose(1, 0, 2)),
            "wv": np.ascontiguousarray(
                wv_c.reshape(NDC, 128, DH).transpose(1, 0, 2)),
            "wo": np.ascontiguousarray(                 # [128, G, D]
                wo_c.reshape(G, 128, D).transpose(1, 0, 2)),
            "cosT": cosT,
            "sinT": sinT,
        }
        if mask_pre is not None:
            m["maskm"] = mask_pre
        maps.append(m)
    return active, mt, maps


def kernel(hidden_states, mask, Wq, Wk, Wv, Wo):
    from concourse.bass_utils import run_bass_kernel_spmd

    active, mt, maps = _in_maps(hidden_states, mask, Wq, Wk, Wv, Wo)
    key = tuple(tuple(lst) for lst in active)
    if key not in _CACHE:
        _CACHE[key] = _build(active, 0 if mt is None else mt.shape[0] // 128)
    nc = _CACHE[key]

    res = run_bass_kernel_spmd(nc, maps, list(range(N_CORES)))
    acc = np.zeros((S, D), dtype=np.float64)
    for c in range(N_CORES):
        acc += res.results[c]["out"]
    return acc.astype(np.float32).reshape(1, S, D)


# revision 14
# speedup vs baseline: 1.2855x; 1.0001x over previous
"""GQA multi-head attention (RoPE + tanh softcap + causal mask) on 8 TRN2 cores.

Sharding: tensor-parallel over the 8 kv-head groups (1 kv head + its 4 q heads
per core).  Each core computes its Q/K/V projections from the full hidden
states, runs attention for its 4 q heads, and produces a partial output
through its row-slice of Wo; the host sums the 8 partials.

v2 layout/schedule (vs the fp32 two-phase baseline):
  - all matmul operands are bf16 (PSUM accumulation stays fp32; softmax
    logits/tanh stay fp32).  Halves DMA + SBUF traffic and doubles DVE
    throughput on 16-bit elementwise work.  Measured end-to-end rel err
    ~4e-3 vs the 2e-2 gate.
  - single fused per-block pipeline: project block n (two 3-output passes
    over resident hs tiles) -> attention for q-block n over kv chunks
    0..n -> output projection rows of block n.  The tensor engine always
    has matmul work queued, so the HAM clock gate stays at 8/8 (the old
    kernel ran at 4/8 for 75% of its span).
  - softmax denominators accumulate on the PE: a per-chunk [1,512]
    ones-matmul rides the same PSUM accumulation pattern as A@V, replacing
    the serial vector-engine running-sum chain.
  - 1/denominator via the custom-DVE reciprocal_approx_fast (~5x faster
    than the 8-cycle/element iterative divide).
  - V tiles are transposed with the DMA crossbar (dma_start_transpose)
    instead of PE transposes, freeing PE time and a PSUM bank.
"""

import numpy as np

S, D, DH = 2048, 4096, 128
HQ, HKV = 32, 8
G = HQ // HKV            # q heads per core
N_CORES = 8
MULT = 0.08838834764831845
SOFTCAP = 30.0
ROPE_BASE = 10000.0
BLK = 512                # seq block
NB = S // BLK            # 4 seq blocks
NCH = S // 128           # 16 kcol chunks
NDC = D // 128           # 32 contraction chunks for projections
HDC = NDC // 2           # 16 d-chunks per hs half-block tile

_CACHE = {}


def _classify_mask(mask):
    """Per (qblock, kchunk): skip (all masked), plain (all visible), or
    mixed (transposed {0,1} tile, deduped).  active[n] = ordered
    [(chunk, slot)], slot -1 for plain; mtiles packed [n_uniq*128, BLK]."""
    m = np.asarray(mask).reshape(S, S)
    active = []
    mtiles = []
    seen = {}
    for n in range(NB):
        rows = m[n * BLK:(n + 1) * BLK]
        lst = []
        for c in range(NCH):
            sub = rows[:, c * 128:(c + 1) * 128]
            if not sub.any():
                continue
            if sub.all():
                lst.append((c, -1))
            else:
                t = np.ascontiguousarray(sub.T).astype(np.float32)
                key = t.tobytes()
                if key not in seen:
                    seen[key] = len(mtiles)
                    mtiles.append(t)
                lst.append((c, seen[key]))
        active.append(lst)
    mt = (np.concatenate([t.reshape(128, BLK) for t in mtiles], axis=0)
          if mtiles else None)
    return active, mt


def _build(active, n_uniq):
    import concourse.bacc as bacc
    import concourse.mybir as mybir
    from concourse import tile
    from contextlib import ExitStack

    fp32 = mybir.dt.float32
    bf16 = mybir.dt.bfloat16
    AF = mybir.ActivationFunctionType

    nc = bacc.Bacc("TRN2", target_bir_lowering=False, debug=False,
                   enable_asserts=True, num_devices=N_CORES)
    # all inputs host-pre-arranged partition-major so every DMA moves
    # multi-KB contiguous segments per partition (row-major slices of the
    # natural layouts produce 256B-1KB packets and a ~55us DMA-bound start)
    hsT_r = nc.dram_tensor("hsT", [128, NB * 2, HDC, BLK], bf16,
                           kind="ExternalInput").ap()
    wq_r = nc.dram_tensor("wq", [128, G, NDC, DH], bf16,
                          kind="ExternalInput").ap()
    wk_r = nc.dram_tensor("wk", [128, NDC, DH], bf16,
                          kind="ExternalInput").ap()
    wv_r = nc.dram_tensor("wv", [128, NDC, DH], bf16,
                          kind="ExternalInput").ap()
    wo_r = nc.dram_tensor("wo", [128, G, D], bf16,
                          kind="ExternalInput").ap()
    cosT = nc.dram_tensor("cosT", [DH, S], bf16, kind="ExternalInput").ap()
    sinT = nc.dram_tensor("sinT", [DH, S], bf16, kind="ExternalInput").ap()
    maskm = (nc.dram_tensor("maskm", [128, n_uniq, BLK], bf16,
                            kind="ExternalInput").ap() if n_uniq else None)
    out = nc.dram_tensor("out", [S, D], fp32, kind="ExternalOutput").ap()

    with tile.TileContext(nc) as tc, ExitStack() as top:
        persist = top.enter_context(tc.tile_pool(name="persist", bufs=1))
        # weights: per-head wq tiles so the first matmuls don't wait on the
        # whole 4MB load
        wq_sb = [persist.tile([128, NDC, DH], bf16, tag=f"wq{h}",
                              name=f"wq{h}") for h in range(G)]
        wk_sb = persist.tile([128, NDC, DH], bf16, tag="wk", name="wk")
        wv_sb = persist.tile([128, NDC, DH], bf16, tag="wv", name="wv")
        wo_sb = persist.tile([128, G, D], bf16, tag="wo", name="wo_sb")
        cos_sb = persist.tile([DH, S], bf16, tag="cos", name="cos")
        sin_sb = persist.tile([DH, S], bf16, tag="sin", name="sin")
        ones_bf = persist.tile([128, 1], bf16, tag="ones", name="ones")
        mask_sb = (persist.tile([128, n_uniq, BLK], bf16, tag="masks",
                                name="masks") if n_uniq else None)
        qT = [[persist.tile([DH, BLK], bf16, tag=f"qT{h}_{n}",
                            name=f"qT{h}_{n}") for n in range(NB)]
              for h in range(G)]
        kT = [persist.tile([DH, BLK], bf16, tag=f"kT{n}", name=f"kT{n}")
              for n in range(NB)]
        vnat = [persist.tile([128, DH], bf16, tag=f"vnat{c}", name=f"vnat{c}")
                for c in range(NCH)]
        attnT = [[persist.tile([DH, BLK], bf16, tag=f"attnT{h}_{n}",
                               name=f"attnT{h}_{n}") for n in range(NB)]
                 for h in range(G)]

        # weight loads: only wq0/wq1 (+ the first hs tiles, loaded in
        # proj_block) are needed immediately; defer the rest with staggered
        # scheduler timestamps so the critical first loads get the full DMA
        # bandwidth instead of fair-sharing with a 13MB flood
        nc.sync.dma_start(wq_sb[0][:], wq_r[:, 0])
        nc.vector.memset(ones_bf[:], 1.0)
        with tc.tile_wait_until(0.005):
            nc.scalar.dma_start(wq_sb[1][:], wq_r[:, 1])
        with tc.tile_wait_until(0.012):
            for h in range(2, G):
                nc.scalar.dma_start(wq_sb[h][:], wq_r[:, h])
        with tc.tile_wait_until(0.02):
            nc.sync.dma_start(wk_sb[:], wk_r[:])
            nc.sync.dma_start(wv_sb[:], wv_r[:])
            nc.scalar.dma_start(cos_sb[:], cosT[:])
            nc.scalar.dma_start(sin_sb[:], sinT[:])
        with tc.tile_wait_until(0.028):
            if n_uniq:
                nc.scalar.dma_start(mask_sb[:], maskm[:])
        with tc.tile_wait_until(0.036):
            for g in range(8):
                nc.gpsimd.dma_start(wo_sb[:, :, g * BLK:(g + 1) * BLK],
                                    wo_r[:, :, g * BLK:(g + 1) * BLK])

        # working pools
        hsp = top.enter_context(tc.tile_pool(name="hs", bufs=2))
        pps = top.enter_context(tc.tile_pool(name="projps", bufs=2,
                                             space="PSUM"))
        rawp = top.enter_context(tc.tile_pool(name="raw", bufs=2))
        rotp = top.enter_context(tc.tile_pool(name="rot", bufs=2))
        tmpp = top.enter_context(tc.tile_pool(name="tmp", bufs=2))
        vtp = top.enter_context(tc.tile_pool(name="vt", bufs=2))
        ttp = top.enter_context(tc.tile_pool(name="tt", bufs=2))
        wtp = top.enter_context(tc.tile_pool(name="wt", bufs=3))
        dsp = top.enter_context(tc.tile_pool(name="dns", bufs=2))
        bcp = top.enter_context(tc.tile_pool(name="bc", bufs=2))
        osb = top.enter_context(tc.tile_pool(name="osb", bufs=3))
        qkps = top.enter_context(tc.tile_pool(name="qkps", bufs=2,
                                              space="PSUM"))
        avps = top.enter_context(tc.tile_pool(name="avps", bufs=1,
                                              space="PSUM"))
        dnps = top.enter_context(tc.tile_pool(name="dnps", bufs=1,
                                              space="PSUM"))
        wops = top.enter_context(tc.tile_pool(name="wops", bufs=2,
                                              space="PSUM"))

        def wo_block(n):
            for j in range(BLK // 128):
                s = n * (BLK // 128) + j
                for nn2 in range(D // (2 * BLK)):
                    ot = osb.tile([128, 2 * BLK], fp32, tag="ot", name="ot")
                    for half in range(2):
                        nn = nn2 * 2 + half
                        pso = wops.tile([128, BLK], fp32, tag="wop",
                                        name="wop")
                        for h in range(G):
                            nc.tensor.matmul(
                                pso[:], attnT[h][n][:, j * 128:(j + 1) * 128],
                                wo_sb[:, h, nn * BLK:(nn + 1) * BLK],
                                start=(h == 0), stop=(h == G - 1),
                                skip_group_check=True)
                        nc.vector.tensor_copy(
                            ot[:, half * BLK:(half + 1) * BLK], pso[:])
                    nc.gpsimd.dma_start(
                        out[s * 128:(s + 1) * 128,
                            nn2 * 2 * BLK:(nn2 + 1) * 2 * BLK], ot[:])

        def rope_evict(ps, dest, sl):
            raw = rawp.tile([128, BLK], bf16, tag="raw", name="raw")
            nc.scalar.copy(raw[:], ps[:])
            rot = rotp.tile([128, BLK], bf16, tag="rot", name="rot")
            nc.sync.dma_start(rot[0:64, :], raw[64:128, :])
            nc.sync.dma_start(rot[64:128, :], raw[0:64, :])
            tmp = tmpp.tile([128, BLK], bf16, tag="tmp", name="tmp")
            nc.vector.tensor_mul(tmp[:], raw[:], cos_sb[:, sl])
            nc.vector.tensor_mul(rot[:], rot[:], sin_sb[:, sl])
            nc.vector.tensor_add(dest[:], tmp[:], rot[:])

        def hs_load(n, split=False):
            hs_t = [hsp.tile([128, HDC, BLK], bf16, tag="hs", name="hs_t")
                    for _ in range(2)]
            for t in range(2):
                eng = nc.scalar if (split and t == 0) else nc.sync
                eng.dma_start(hs_t[t][:], hsT_r[:, n * 2 + t])
            return hs_t

        def proj_pass(hs_t, w0):
            ps0 = pps.tile([128, BLK], fp32, tag="projps", name="projps")
            for d in range(NDC):
                h_ap = hs_t[d // HDC][:, d % HDC, :]
                nc.tensor.matmul(ps0[:], w0[:, d, :], h_ap,
                                 start=(d == 0), stop=(d == NDC - 1))
            return ps0

        def attention(n):
            pairs = [active[n][i:i + 2] for i in range(0, len(active[n]), 2)]
            n_mm = len(active[n])
            for h in range(G):
                av = avps.tile([128, BLK], fp32, tag="av", name="av")
                dn = dnps.tile([1, BLK], fp32, tag="dn", name="dn")
                mm_i = 0
                for pair in pairs:
                    w2 = len(pair) * BLK
                    tt = ttp.tile([128, 2 * BLK], fp32, tag="tt", name="tt")
                    for i, (c, slot) in enumerate(pair):
                        qk = qkps.tile([128, BLK], fp32, tag="qk", name="qk")
                        nc.tensor.matmul(
                            qk[:],
                            kT[c // 4][:, (c % 4) * 128:(c % 4 + 1) * 128],
                            qT[h][n][:], start=True, stop=True)
                        nc.scalar.activation(
                            tt[:, i * BLK:(i + 1) * BLK], qk[:],
                            AF.Tanh, scale=1.0 / SOFTCAP)
                    wt = wtp.tile([128, 2 * BLK], bf16, tag="wt", name="wt")
                    nc.scalar.activation(wt[:, :w2], tt[:, :w2], AF.Exp,
                                         scale=SOFTCAP)
                    for i, (c, slot) in enumerate(pair):
                        if slot >= 0:
                            nc.vector.tensor_mul(
                                wt[:, i * BLK:(i + 1) * BLK],
                                wt[:, i * BLK:(i + 1) * BLK],
                                mask_sb[:, slot, :])
                    for i, (c, slot) in enumerate(pair):
                        wt_ap = wt[:, i * BLK:(i + 1) * BLK]
                        nc.tensor.matmul(av[:], vnat[c][:], wt_ap,
                                         start=(mm_i == 0),
                                         stop=(mm_i == n_mm - 1),
                                         skip_group_check=True)
                        nc.tensor.matmul(dn[:], ones_bf[:], wt_ap,
                                         start=(mm_i == 0),
                                         stop=(mm_i == n_mm - 1),
                                         skip_group_check=True)
                        mm_i += 1
                dns = dsp.tile([1, BLK], fp32, tag="dns", name="dns")
                nc.vector.reciprocal_approx_fast(dns[:], dn[:])
                bc = bcp.tile([128, BLK], fp32, tag="bc", name="bc")
                nc.gpsimd.partition_broadcast(bc[:], dns[:])
                nc.vector.tensor_mul(attnT[h][n][:], av[:], bc[:])

        def proj_block(n):
            sl = slice(n * BLK, (n + 1) * BLK)
            hs_t = hs_load(n)
            for h in range(G):
                rope_evict(proj_pass(hs_t, wq_sb[h]), qT[h][n], sl)
            rope_evict(proj_pass(hs_t, wk_sb), kT[n], sl)
            psv = proj_pass(hs_t, wv_sb)
            # V: evict to bf16, transpose chunks with the DMA crossbar
            vt = vtp.tile([128, BLK], bf16, tag="vt", name="vt")
            nc.scalar.copy(vt[:], psv[:])
            for j in range(BLK // 128):
                c = n * (BLK // 128) + j
                nc.sync.dma_start_transpose(
                    vnat[c][:], vt[:, j * 128:(j + 1) * 128])

        # Schedule: attention(n) directly after its projection (the
        # scheduler overlaps it with proj(n+1)); each block's output
        # projection is emitted one attention later so its matmuls fill the
        # next attention's pipeline gaps.
        for n in range(NB):
            proj_block(n)
            attention(n)
            if n >= 1:
                wo_block(n - 1)
        wo_block(NB - 1)

    nc.compile()
    return nc


def _rope_tables():
    j = np.arange(0, DH, 2, dtype=np.float32)
    inv = np.float32(1.0) / (np.float32(ROPE_BASE) ** (j / np.float32(DH)))
    t = np.arange(S, dtype=np.float32)
    phase = t[:, None] * inv[None, :]          # [S, 64] fp32 like reference
    cos = np.cos(phase).astype(np.float32)     # [S, 64]
    sin = np.sin(phase).astype(np.float32)
    cosT = np.concatenate([cos.T, cos.T], axis=0)              # [128, S]
    sinT = np.concatenate([-sin.T, sin.T], axis=0)             # sign-folded
    return np.ascontiguousarray(cosT), np.ascontiguousarray(sinT)


def _in_maps(hidden_states, mask, Wq, Wk, Wv, Wo):
    import ml_dtypes
    bf = ml_dtypes.bfloat16
    hs = np.asarray(hidden_states, dtype=np.float32).reshape(S, D)
    Wq = np.asarray(Wq, dtype=np.float32)
    Wk = np.asarray(Wk, dtype=np.float32)
    Wv = np.asarray(Wv, dtype=np.float32)
    Wo = np.asarray(Wo, dtype=np.float32)
    active, mt = _classify_mask(mask)
    # partition-major pre-arrangements (see _build comment): index d becomes
    # (chunk c, partition p) with p fastest; tiles made contiguous
    hsT = hs.T.astype(bf)                       # [D, S]
    hs_pre = np.ascontiguousarray(              # [128, NB*2, HDC, BLK]
        hsT.reshape(2, HDC, 128, NB, BLK).transpose(2, 3, 0, 1, 4)
        .reshape(128, NB * 2, HDC, BLK))
    cosT, sinT = _rope_tables()
    cosT = cosT.astype(bf)
    sinT = sinT.astype(bf)
    mask_pre = (np.ascontiguousarray(
        mt.astype(bf).reshape(-1, 128, BLK).transpose(1, 0, 2))
        if mt is not None else None)
    maps = []
    for c in range(N_CORES):
        wq_c = (Wq[:, c * G * DH:(c + 1) * G * DH]
                * np.float32(MULT)).astype(bf)          # [D, G*DH]
        wk_c = Wk[:, c * DH:(c + 1) * DH].astype(bf)    # [D, DH]
        wv_c = Wv[:, c * DH:(c + 1) * DH].astype(bf)
        wo_c = Wo[c * G * DH:(c + 1) * G * DH, :].astype(bf)  # [G*DH, D]
        m = {
            "hsT": hs_pre,
            "wq": np.ascontiguousarray(                 # [128, G, NDC, DH]
                wq_c.reshape(NDC, 128, G, DH).transpose(1, 2, 0, 3)),
            "wk": np.ascontiguousarray(                 # [128, NDC, DH]
                wk_c.reshape(NDC, 128, DH).transpose(1, 0, 2)),
            "wv": np.ascontiguousarray(
                wv_c.reshape(NDC, 128, DH).transpose(1, 0, 2)),
            "wo": np.ascontiguousarray(                 # [128, G, D]
                wo_c.reshape(G, 128, D).transpose(1, 0, 2)),
            "cosT": cosT,
            "sinT": sinT,
        }
        if mask_pre is not None:
            m["maskm"] = mask_pre
        maps.append(m)
    return active, mt, maps


def kernel(hidden_states, mask, Wq, Wk, Wv, Wo):
    from concourse.bass_utils import run_bass_kernel_spmd

    active, mt, maps = _in_maps(hidden_states, mask, Wq, Wk, Wv, Wo)
    key = tuple(tuple(lst) for lst in active)
    if key not in _CACHE:
        _CACHE[key] = _build(active, 0 if mt is None else mt.shape[0] // 128)
    nc = _CACHE[key]

    res = run_bass_kernel_spmd(nc, maps, list(range(N_CORES)))
    acc = np.zeros((S, D), dtype=np.float64)
    for c in range(N_CORES):
        acc += res.results[c]["out"]
    return acc.astype(np.float32).reshape(1, S, D)
